# revision 1
# baseline (speedup 1.0000x reference)
"""Trainium2 Bass kernel for nn_CalibrationNetwork (MoE routing over 12 judges).

Strategy: shared + judge-specific weights are pre-summed on the host into 12
effective per-judge MLPs (the einsum+take_along_axis in the reference is just
"route each sample through the MLP of its judge").  Samples are sorted by
judge id on the host, each judge's slots padded to a fixed capacity 2*Cc, and
the resulting 24 fixed-size chunks (2 per judge) are dealt 3-per-core to the 8
NeuronCores.  Every core runs the same static Bass/Tile program: for each of
its 3 chunks, dense fp32 matmuls (layer1 K=36, layer2 K=256, heads K=256) with
relu/bias fused into the PSUM-evacuation, then a grouped softmax over the 7x5
head logits, all per-judge weights DMA'd per chunk.  Outputs are scattered
back to the original sample order on the host.
"""

import os
import sys

import numpy as np

for _p in ("/opt/trn_rl_repo", "/root/.axon_site/_ro/trn_rl_repo"):
    if os.path.isdir(_p) and _p not in sys.path:
        sys.path.insert(0, _p)

B, D, H1, H2, J, Q, O = 32768, 35, 256, 256, 12, 7, 5
NCORES = 8
SEG = 3                    # chunks per core
NCHUNKS = NCORES * SEG     # 24 = 2 chunks per judge

USE_F32R = False           # PE fast-fp32 mode for the big matmuls
TRACE = False              # set True in test harness to collect NTFF profile
LAST_RESULTS = None        # BassKernelResults of the last run (for test.py)

_PROG_CACHE = {}


def _build_program(Cc, use_f32r):
    import concourse.bass as bass
    import concourse.tile as tile
    from concourse import bacc, mybir

    f32 = mybir.dt.float32
    f32r = mybir.dt.float32r
    AF = mybir.ActivationFunctionType
    ALU = mybir.AluOpType

    NT = Cc // 512            # 512-wide n-tiles per chunk
    TS = Cc // 128            # 128-row batch subtiles per chunk
    QO = Q * O

    nc = bacc.Bacc(None, target_bir_lowering=False, debug=False)

    xt_d = nc.dram_tensor("xt", [D + 1, SEG * Cc], f32, kind="ExternalInput")
    a1_d = nc.dram_tensor("a1t", [SEG, D + 1, H1], f32, kind="ExternalInput")
    a2_d = nc.dram_tensor("a2t", [SEG, 128, 2, H2], f32, kind="ExternalInput")
    b2_d = nc.dram_tensor("b2", [SEG, 128, 2], f32, kind="ExternalInput")
    av_d = nc.dram_tensor("avt", [SEG, 128, 2, QO], f32, kind="ExternalInput")
    bv_d = nc.dram_tensor("bv", [SEG, QO], f32, kind="ExternalInput")
    out_d = nc.dram_tensor("out", [SEG * Cc, QO], f32, kind="ExternalOutput")

    def mm(ap):
        return ap.bitcast(f32r) if use_f32r else ap

    with tile.TileContext(nc) as tc:
        with (
            tc.tile_pool(name="xp", bufs=1) as xp,
            tc.tile_pool(name="wp", bufs=2) as wp,
            tc.tile_pool(name="zp", bufs=2) as zp,
            tc.tile_pool(name="op", bufs=3) as op_,
            tc.tile_pool(name="ps", bufs=2, space="PSUM") as ps,
        ):
            xt = xp.tile([D + 1, SEG * Cc], f32)
            nc.sync.dma_start(xt[:], xt_d[:])

            for s in range(SEG):
                a1 = wp.tile([D + 1, H1], f32, tag="a1")
                nc.sync.dma_start(a1[:], a1_d[s])
                a2 = wp.tile([128, 2, H2], f32, tag="a2")
                nc.sync.dma_start(a2[:], a2_d[s])
                b2 = wp.tile([128, 2], f32, tag="b2")
                nc.sync.dma_start(b2[:], b2_d[s])
                av = wp.tile([128, 2, QO], f32, tag="av")
                nc.sync.dma_start(av[:], av_d[s])
                bv = wp.tile([128, QO], f32, tag="bv")
                bsrc = bv_d[s]
                nc.sync.dma_start(
                    bv[:],
                    bass.AP(
                        tensor=bsrc.tensor,
                        offset=bsrc.offset,
                        ap=[[0, 128]] + [list(d) for d in bsrc.ap],
                    ),
                )

                z1 = zp.tile([128, 2, Cc], f32, tag="z1")
                z2 = zp.tile([128, 2, Cc], f32, tag="z2")

                # ---- layer 1: z1 = relu(xb @ A1eff.T), bias folded in ones col
                for m in range(2):
                    for n in range(NT):
                        p1 = ps.tile([128, 512], f32, tag="l1")
                        nc.tensor.matmul(
                            p1[:],
                            mm(a1[:, m * 128 : (m + 1) * 128]),
                            mm(xt[:, s * Cc + n * 512 : s * Cc + (n + 1) * 512]),
                            start=True,
                            stop=True,
                        )
                        nc.scalar.activation(
                            z1[:, m, n * 512 : (n + 1) * 512], p1[:], AF.Relu
                        )

                # ---- layer 2: z2 = relu(z1b @ A2eff.T + b2)
                for m in range(2):
                    for n in range(NT):
                        p2 = ps.tile([128, 512], f32, tag="l2")
                        for k in range(2):
                            nc.tensor.matmul(
                                p2[:],
                                mm(a2[:, k, m * 128 : (m + 1) * 128]),
                                mm(z1[:, k, n * 512 : (n + 1) * 512]),
                                start=(k == 0),
                                stop=(k == 1),
                            )
                        nc.vector.tensor_scalar(
                            out=z2[:, m, n * 512 : (n + 1) * 512],
                            in0=p2[:],
                            scalar1=b2[:, m : m + 1],
                            scalar2=0.0,
                            op0=ALU.add,
                            op1=ALU.max,
                        )

                # ---- heads + softmax, in groups of hg batch-subtiles
                hg = min(TS, 512 // QO)
                for g0 in range(0, TS, hg):
                    cnt = min(hg, TS - g0)
                    ph = ps.tile([128, hg, QO], f32, tag="hd")
                    for t in range(cnt):
                        for k in range(2):
                            nc.tensor.matmul(
                                ph[:, t, :],
                                mm(z2[:, k, (g0 + t) * 128 : (g0 + t + 1) * 128]),
                                mm(av[:, k, :]),
                                start=(k == 0),
                                stop=(k == 1),
                            )
                    texp = op_.tile([128, hg, QO], f32, tag="texp")
                    nc.vector.tensor_tensor(
                        texp[:, :cnt],
                        ph[:, :cnt],
                        bv[:, None, :].broadcast_to([128, cnt, QO]),
                        ALU.add,
                    )
                    nc.scalar.activation(texp[:, :cnt], texp[:, :cnt], AF.Exp)
                    sums = op_.tile([128, hg, Q], f32, tag="sums")
                    nc.vector.reduce_sum(
                        out=sums[:, :cnt],
                        in_=texp[:, :cnt].rearrange("p a (q o) -> p a q o", o=O),
                        axis=mybir.AxisListType.X,
                    )
                    nc.vector.reciprocal(sums[:, :cnt], sums[:, :cnt])
                    outt = op_.tile([128, hg, QO], f32, tag="outt")
                    nc.vector.tensor_tensor(
                        outt[:, :cnt].rearrange("p a (q o) -> p a q o", o=O),
                        texp[:, :cnt].rearrange("p a (q o) -> p a q o", o=O),
                        sums[:, :cnt, :, None].broadcast_to([128, cnt, Q, O]),
                        ALU.mult,
                    )
                    nc.sync.dma_start(
                        out_d[s * Cc + g0 * 128 : s * Cc + (g0 + cnt) * 128, :].rearrange(
                            "(t p) o -> p t o", p=128
                        ),
                        outt[:, :cnt],
                    )

    nc.compile()
    return nc


def _get_program(Cc, use_f32r):
    key = (Cc, use_f32r)
    if key not in _PROG_CACHE:
        _PROG_CACHE[key] = _build_program(Cc, use_f32r)
    return _PROG_CACHE[key]


def kernel(**inputs):
    global LAST_RESULTS
    x = np.ascontiguousarray(np.asarray(inputs["x"], dtype=np.float32))
    ids = np.asarray(inputs["judge_ids"]).astype(np.int64).ravel()
    W1_w = np.asarray(inputs["W1_w"], np.float32)
    W1_b = np.asarray(inputs["W1_b"], np.float32)
    W2_w = np.asarray(inputs["W2_w"], np.float32)
    W2_b = np.asarray(inputs["W2_b"], np.float32)
    W1a_w = np.asarray(inputs["W1a_w"], np.float32)
    W1a_b = np.asarray(inputs["W1a_b"], np.float32)
    W2a_w = np.asarray(inputs["W2a_w"], np.float32)
    W2a_b = np.asarray(inputs["W2a_b"], np.float32)
    V_w = np.asarray(inputs["V_w"], np.float32)
    V_b = np.asarray(inputs["V_b"], np.float32)
    Va_w = np.asarray(inputs["Va_w"], np.float32)
    Va_b = np.asarray(inputs["Va_b"], np.float32)

    Bx = x.shape[0]
    cnts = np.bincount(ids, minlength=J)
    Cc = 1536
    mx = int(cnts.max())
    if 2 * Cc < mx:
        Cc = ((mx + 1) // 2 + 511) // 512 * 512

    # effective per-judge weights (shared + judge-specific, biases folded)
    A1 = (W1_w[None] + W1a_w).copy()                      # (J, H1, D+1)
    A1[:, :, D] += W1_b[None] + W1a_b
    A2 = W2_w[None] + W2a_w                               # (J, H2, H1+1)
    b2 = A2[:, :, H1] + W2_b[None] + W2a_b                # (J, H2)
    A2c = A2[:, :, :H1]                                   # (J, H2, H1)
    AV = (V_w[None] + Va_w).reshape(J, Q * O, H2 + 1)
    bV = (AV[:, :, H2] + (V_b[None] + Va_b).reshape(J, Q * O)).astype(np.float32)
    AVc = AV[:, :, :H2]

    # SBUF layouts
    a1sb = np.ascontiguousarray(np.transpose(A1, (0, 2, 1)))  # (J, 36, 256)
    a2sb = np.ascontiguousarray(
        np.transpose(A2c.reshape(J, H2, 2, 128), (0, 3, 2, 1))
    )  # (J, 128, 2, 256): [j,p,k,m] = A2c[j][m, k*128+p]
    b2sb = np.ascontiguousarray(np.transpose(b2.reshape(J, 2, 128), (0, 2, 1)))
    avsb = np.ascontiguousarray(
        np.transpose(AVc.reshape(J, Q * O, 2, 128), (0, 3, 2, 1))
    )  # (J, 128, 2, 35)

    # slot -> sample map: judge j owns slots [j*2Cc, (j+1)*2Cc)
    order = np.argsort(ids, kind="stable")
    slot2samp = np.full(NCHUNKS * Cc, -1, np.int64)
    pos = 0
    for j in range(J):
        k = int(cnts[j])
        slot2samp[j * 2 * Cc : j * 2 * Cc + k] = order[pos : pos + k]
        pos += k
    chunk_judge = np.repeat(np.arange(J), 2)

    in_maps = []
    core_meta = []
    for c in range(NCORES):
        sl = slot2samp[c * SEG * Cc : (c + 1) * SEG * Cc]
        valid = sl >= 0
        Xc = np.zeros((SEG * Cc, D + 1), np.float32)
        Xc[valid, :D] = x[sl[valid]]
        Xc[:, D] = 1.0
        js = chunk_judge[c * SEG : (c + 1) * SEG]
        in_maps.append(
            {
                "xt": np.ascontiguousarray(Xc.T),
                "a1t": np.ascontiguousarray(a1sb[js]),
                "a2t": np.ascontiguousarray(a2sb[js]),
                "b2": np.ascontiguousarray(b2sb[js]),
                "avt": np.ascontiguousarray(avsb[js]),
                "bv": np.ascontiguousarray(bV[js]),
            }
        )
        core_meta.append((sl, valid))

    nc = _get_program(Cc, USE_F32R)
    from concourse.bass_utils import run_bass_kernel_spmd

    res = run_bass_kernel_spmd(
        nc,
        in_maps,
        core_ids=list(range(NCORES)),
        trace=TRACE,
    )
    LAST_RESULTS = res

    full = np.zeros((Bx, Q, O), np.float32)
    for c in range(NCORES):
        oc = np.asarray(res.results[c]["out"])
        sl, valid = core_meta[c]
        full[sl[valid]] = oc[valid].reshape(-1, Q, O)
    return full


# revision 8
# speedup vs baseline: 1.5496x; 1.5496x over previous
"""Trainium2 Bass kernel for nn_CalibrationNetwork (MoE routing over 12 judges).

Strategy: shared + judge-specific weights are pre-summed on the host into 12
effective per-judge MLPs (the einsum+take_along_axis in the reference is just
"route each sample through the MLP of its judge").  Samples are sorted by
judge id on the host, each judge's slots padded to a fixed capacity 2*Cc, and
the resulting 24 fixed-size chunks (2 per judge) are dealt 3-per-core to the 8
NeuronCores.  Every core runs the same static Bass/Tile program: for each of
its 3 chunks, dense fp32 matmuls (layer1 K=36, layer2 K=256, heads K=256) with
relu/bias fused into the PSUM-evacuation, then a grouped softmax over the 7x5
head logits, all per-judge weights DMA'd per chunk.  Outputs are scattered
back to the original sample order on the host.
"""

import os
import sys

import numpy as np

for _p in ("/opt/trn_rl_repo", "/root/.axon_site/_ro/trn_rl_repo"):
    if os.path.isdir(_p) and _p not in sys.path:
        sys.path.insert(0, _p)

B, D, H1, H2, J, Q, O = 32768, 35, 256, 256, 12, 7, 5
NCORES = 8
SEG = 3                    # chunks per core
NCHUNKS = NCORES * SEG     # 24 = 2 chunks per judge

USE_F32R = False           # PE fast-fp32 mode for the big matmuls
TRACE = False              # set True in test harness to collect NTFF profile
LAST_RESULTS = None        # BassKernelResults of the last run (for test.py)

_PROG_CACHE = {}


def _build_program(Cc, use_f32r):
    import concourse.bass as bass
    import concourse.tile as tile
    from concourse import bacc, mybir

    f32 = mybir.dt.float32
    f32r = mybir.dt.float32r
    fmm = f32r if use_f32r else f32   # dtype of matmul operands
    AF = mybir.ActivationFunctionType
    ALU = mybir.AluOpType

    NT = Cc // 512            # 512-wide n-tiles per chunk
    TS = Cc // 128            # 128-row batch subtiles per chunk
    QO = Q * O
    QOp = QO + 1              # head out dim padded even (f32r needs even N)

    nc = bacc.Bacc(None, target_bir_lowering=False, debug=False)

    xt_d = nc.dram_tensor("xt", [D + 1, SEG * Cc], fmm, kind="ExternalInput")
    a1_d = nc.dram_tensor("a1t", [SEG, D + 1, H1], fmm, kind="ExternalInput")
    a2_d = nc.dram_tensor("a2t", [SEG, 128, 2, H2], fmm, kind="ExternalInput")
    b2_d = nc.dram_tensor("b2", [SEG, 128, 2], f32, kind="ExternalInput")
    av_d = nc.dram_tensor("avt", [SEG, 128, 2, QOp], fmm, kind="ExternalInput")
    bv_d = nc.dram_tensor("bv", [SEG, QO], f32, kind="ExternalInput")
    out_d = nc.dram_tensor("out", [SEG * Cc, QO], f32, kind="ExternalOutput")

    def mm(ap):
        return ap

    with tile.TileContext(nc) as tc:
        with (
            tc.tile_pool(name="xp", bufs=1) as xp,
            tc.tile_pool(name="wp", bufs=2) as wp,
            tc.tile_pool(name="zp", bufs=2) as zp,
            tc.tile_pool(name="op", bufs=3) as op_,
            tc.tile_pool(name="ps", bufs=2, space="PSUM") as ps,
        ):
            xt = xp.tile([D + 1, SEG * Cc], fmm)
            nc.sync.dma_start(xt[:], xt_d[:])

            for s in range(SEG):
                a1 = wp.tile([D + 1, H1], fmm, tag="a1")
                nc.sync.dma_start(a1[:], a1_d[s])
                a2 = wp.tile([128, 2, H2], fmm, tag="a2")
                nc.sync.dma_start(a2[:], a2_d[s])
                b2 = wp.tile([128, 2], f32, tag="b2")
                nc.sync.dma_start(b2[:], b2_d[s])
                av = wp.tile([128, 2, QOp], fmm, tag="av")
                nc.sync.dma_start(av[:], av_d[s])
                bv = wp.tile([128, QO], f32, tag="bv")
                bsrc = bv_d[s]
                nc.sync.dma_start(
                    bv[:],
                    bass.AP(
                        tensor=bsrc.tensor,
                        offset=bsrc.offset,
                        ap=[[0, 128]] + [list(d) for d in bsrc.ap],
                    ),
                )

                z1 = zp.tile([128, 2, Cc], fmm, tag="z1")
                z2 = zp.tile([128, 2, Cc], fmm, tag="z2")

                # ---- layer 1: z1 = relu(xb @ A1eff.T), bias folded in ones col
                for m in range(2):
                    for n in range(NT):
                        p1 = ps.tile([128, 512], f32, tag="l1")
                        nc.tensor.matmul(
                            p1[:],
                            mm(a1[:, m * 128 : (m + 1) * 128]),
                            mm(xt[:, s * Cc + n * 512 : s * Cc + (n + 1) * 512]),
                            start=True,
                            stop=True,
                        )
                        nc.scalar.activation(
                            z1[:, m, n * 512 : (n + 1) * 512], p1[:], AF.Relu
                        )

                # ---- layer 2: z2 = relu(z1b @ A2eff.T + b2)
                for m in range(2):
                    for n in range(NT):
                        p2 = ps.tile([128, 512], f32, tag="l2")
                        for k in range(2):
                            nc.tensor.matmul(
                                p2[:],
                                mm(a2[:, k, m * 128 : (m + 1) * 128]),
                                mm(z1[:, k, n * 512 : (n + 1) * 512]),
                                start=(k == 0),
                                stop=(k == 1),
                            )
                        nc.vector.tensor_scalar(
                            out=z2[:, m, n * 512 : (n + 1) * 512],
                            in0=p2[:],
                            scalar1=b2[:, m : m + 1],
                            scalar2=0.0,
                            op0=ALU.add,
                            op1=ALU.max,
                        )

                # ---- heads + softmax, in groups of hg batch-subtiles
                hg = min(TS, 512 // QOp)
                for g0 in range(0, TS, hg):
                    cnt = min(hg, TS - g0)
                    ph = ps.tile([128, hg, QOp], f32, tag="hd")
                    for t in range(cnt):
                        for k in range(2):
                            nc.tensor.matmul(
                                ph[:, t, :],
                                mm(z2[:, k, (g0 + t) * 128 : (g0 + t + 1) * 128]),
                                mm(av[:, k, :]),
                                start=(k == 0),
                                stop=(k == 1),
                            )
                    texp = op_.tile([128, hg, QOp], f32, tag="texp")
                    nc.vector.tensor_tensor(
                        texp[:, :cnt, :QO],
                        ph[:, :cnt, :QO],
                        bv[:, None, :].broadcast_to([128, cnt, QO]),
                        ALU.add,
                    )
                    nc.scalar.activation(
                        texp[:, :cnt, :QO], texp[:, :cnt, :QO], AF.Exp
                    )
                    sums = op_.tile([128, hg, Q], f32, tag="sums")
                    nc.vector.reduce_sum(
                        out=sums[:, :cnt],
                        in_=texp[:, :cnt, :QO].rearrange("p a (q o) -> p a q o", o=O),
                        axis=mybir.AxisListType.X,
                    )
                    nc.vector.reciprocal(sums[:, :cnt], sums[:, :cnt])
                    outt = op_.tile([128, hg, QO], f32, tag="outt")
                    nc.vector.tensor_tensor(
                        outt[:, :cnt].rearrange("p a (q o) -> p a q o", o=O),
                        texp[:, :cnt, :QO].rearrange("p a (q o) -> p a q o", o=O),
                        sums[:, :cnt, :, None].broadcast_to([128, cnt, Q, O]),
                        ALU.mult,
                    )
                    nc.sync.dma_start(
                        out_d[s * Cc + g0 * 128 : s * Cc + (g0 + cnt) * 128, :].rearrange(
                            "(t p) o -> p t o", p=128
                        ),
                        outt[:, :cnt],
                    )

    nc.compile()
    return nc


def _get_program(Cc, use_f32r):
    key = (Cc, use_f32r)
    if key not in _PROG_CACHE:
        _PROG_CACHE[key] = _build_program(Cc, use_f32r)
    return _PROG_CACHE[key]


def kernel(**inputs):
    global LAST_RESULTS
    x = np.ascontiguousarray(np.asarray(inputs["x"], dtype=np.float32))
    ids = np.asarray(inputs["judge_ids"]).astype(np.int64).ravel()
    W1_w = np.asarray(inputs["W1_w"], np.float32)
    W1_b = np.asarray(inputs["W1_b"], np.float32)
    W2_w = np.asarray(inputs["W2_w"], np.float32)
    W2_b = np.asarray(inputs["W2_b"], np.float32)
    W1a_w = np.asarray(inputs["W1a_w"], np.float32)
    W1a_b = np.asarray(inputs["W1a_b"], np.float32)
    W2a_w = np.asarray(inputs["W2a_w"], np.float32)
    W2a_b = np.asarray(inputs["W2a_b"], np.float32)
    V_w = np.asarray(inputs["V_w"], np.float32)
    V_b = np.asarray(inputs["V_b"], np.float32)
    Va_w = np.asarray(inputs["Va_w"], np.float32)
    Va_b = np.asarray(inputs["Va_b"], np.float32)

    Bx = x.shape[0]
    cnts = np.bincount(ids, minlength=J)
    Cc = 1536
    mx = int(cnts.max())
    if 2 * Cc < mx:
        Cc = ((mx + 1) // 2 + 511) // 512 * 512

    # effective per-judge weights (shared + judge-specific, biases folded)
    A1 = (W1_w[None] + W1a_w).copy()                      # (J, H1, D+1)
    A1[:, :, D] += W1_b[None] + W1a_b
    A2 = W2_w[None] + W2a_w                               # (J, H2, H1+1)
    b2 = A2[:, :, H1] + W2_b[None] + W2a_b                # (J, H2)
    A2c = A2[:, :, :H1]                                   # (J, H2, H1)
    AV = (V_w[None] + Va_w).reshape(J, Q * O, H2 + 1)
    bV = (AV[:, :, H2] + (V_b[None] + Va_b).reshape(J, Q * O)).astype(np.float32)
    AVc = AV[:, :, :H2]

    # SBUF layouts
    a1sb = np.ascontiguousarray(np.transpose(A1, (0, 2, 1)))  # (J, 36, 256)
    a2sb = np.ascontiguousarray(
        np.transpose(A2c.reshape(J, H2, 2, 128), (0, 3, 2, 1))
    )  # (J, 128, 2, 256): [j,p,k,m] = A2c[j][m, k*128+p]
    b2sb = np.ascontiguousarray(np.transpose(b2.reshape(J, 2, 128), (0, 2, 1)))
    avsb = np.transpose(AVc.reshape(J, Q * O, 2, 128), (0, 3, 2, 1))  # (J,128,2,35)
    avsb = np.concatenate(
        [avsb, np.zeros((J, 128, 2, 1), np.float32)], axis=3
    )  # pad head out dim to 36 (f32r even-N requirement)
    avsb = np.ascontiguousarray(avsb)

    # slot -> sample map: judge j owns slots [j*2Cc, (j+1)*2Cc)
    order = np.argsort(ids, kind="stable")
    slot2samp = np.full(NCHUNKS * Cc, -1, np.int64)
    pos = 0
    for j in range(J):
        k = int(cnts[j])
        slot2samp[j * 2 * Cc : j * 2 * Cc + k] = order[pos : pos + k]
        pos += k
    chunk_judge = np.repeat(np.arange(J), 2)

    in_maps = []
    core_meta = []
    for c in range(NCORES):
        sl = slot2samp[c * SEG * Cc : (c + 1) * SEG * Cc]
        valid = sl >= 0
        Xc = np.zeros((SEG * Cc, D + 1), np.float32)
        Xc[valid, :D] = x[sl[valid]]
        Xc[:, D] = 1.0
        js = chunk_judge[c * SEG : (c + 1) * SEG]
        in_maps.append(
            {
                "xt": np.ascontiguousarray(Xc.T),
                "a1t": np.ascontiguousarray(a1sb[js]),
                "a2t": np.ascontiguousarray(a2sb[js]),
                "b2": np.ascontiguousarray(b2sb[js]),
                "avt": np.ascontiguousarray(avsb[js]),
                "bv": np.ascontiguousarray(bV[js]),
            }
        )
        core_meta.append((sl, valid))

    nc = _get_program(Cc, USE_F32R)
    from concourse.bass_utils import run_bass_kernel_spmd

    res = run_bass_kernel_spmd(
        nc,
        in_maps,
        core_ids=list(range(NCORES)),
        trace=TRACE,
    )
    LAST_RESULTS = res

    full = np.zeros((Bx, Q, O), np.float32)
    for c in range(NCORES):
        oc = np.asarray(res.results[c]["out"])
        sl, valid = core_meta[c]
        full[sl[valid]] = oc[valid].reshape(-1, Q, O)
    return full
